# revision 1
# baseline (speedup 1.0000x reference)
"""Additive attention (B=4, Q=128, K=1024, DQ=DK=DV=512, H=256) on 8 TRN2 cores.

ScalarE is the roofline: scores need B*Q*K*H = 134M tanh evaluations and the
activation LUT is the only transcendental unit (1 elem/cycle/lane). Everything
else is arranged to hide under it.

Two SPMD layouts (one graph for all 8 cores, chosen at build time from the
runtime valid_lens — compilation happens inside kernel(), so the graph only
computes scores for k < valid_len):

1. "adaptive" (default): every core takes 16 query rows of EVERY batch, in 4
   groups of 16 slots (one batch per group, extent-sorted: second-smallest
   first for a short prologue, smallest last for a cheap exposed tail). All
   cores share one graph whose per-group k-extent is the exact valid_len, so
   tanh/matvec/projection work shrinks with the data while staying perfectly
   balanced across cores.
2. "full" fallback (valid_lens nearly all-K): core c handles batch c//2,
   query rows 64*(c%2)..+64, full-K scores with an additive -1e6 mask.

Adaptive per-core pipeline:
- kp/qp projections on TensorE (bf16 operands, f32 PSUM), only k < valid_len.
- qp[h,j] + kp[h,k] adds on VectorE (bf16 tensor_scalar_add, 4x mode) into a
  concat buffer; ScalarE runs one bias-free tanh over ~8 slot-halves at a
  time so its 224-cycle per-instruction SBUF bubble amortizes.
- wv-weighted H-reduction as per-slot matvecs on TensorE: lhsT is a 16-column
  sliding window over a zeros|wv buffer, so slot j's score row lands on PSUM
  partition j (matmul output base partitions must be 32-aligned; the zero
  weight columns contribute 0 to other rows).
- Softmax per group straight from PSUM on exactly [:, :valid_len]: no mask
  tensor and no max-subtraction (|score| <= sum|wv| ~ 13, exp is safe in
  f32); exp's accum_out fuses the row-sum; normalization is applied to the
  AV output instead of attn.
- attn^T via PE transpose per 128-column chunk (partial last chunk contracts
  over w < 128 partitions), then per-group AV matmuls accumulate over
  k-chunks of values.
Host-side prep is sharding glue only: per-batch keys^T / values / weights
pre-cast to bf16, plus the zeros|wv window buffer.
"""

import sys

import numpy as np

if "/opt/trn_rl_repo" not in sys.path:
    sys.path.insert(0, "/opt/trn_rl_repo")

import ml_dtypes

B, Q, K = 4, 128, 1024
DQ, DK, DV, H = 512, 512, 512, 256
NEG_INF = -1e6
N_CORES = 8
QPC = (B * Q) // N_CORES  # 64 query rows (slots) per core
HH = H // 128  # h-half count (2)
RPB = QPC // B  # adaptive layout: rows per batch per core (16)

_CACHE = {}


def _build_nc_full():
    import concourse.mybir as mybir
    from concourse import bacc, masks
    from concourse.tile import TileContext

    f32 = mybir.dt.float32
    bf16 = mybir.dt.bfloat16
    AF = mybir.ActivationFunctionType

    nc = bacc.Bacc()

    qT_ext = nc.declare_dram_parameter("queriesT", [DQ, QPC], f32, isOutput=False)
    kT_ext = nc.declare_dram_parameter("keysT", [DK, K], f32, isOutput=False)
    v_ext = nc.declare_dram_parameter("values", [K, DV], f32, isOutput=False)
    wq_ext = nc.declare_dram_parameter("Wq", [DQ, H], f32, isOutput=False)
    wk_ext = nc.declare_dram_parameter("Wk", [DK, H], f32, isOutput=False)
    zwv_ext = nc.declare_dram_parameter("zwv", [HH, 128, 127], bf16, isOutput=False)
    mask_ext = nc.declare_dram_parameter("mask", [1, K], f32, isOutput=False)
    out_ext = nc.declare_dram_parameter("out", [QPC, DV], f32, isOutput=True)

    with TileContext(nc) as tc:
        with (
            tc.tile_pool(name="consts", bufs=1) as consts,
            tc.tile_pool(name="feats", bufs=4) as featp,
            tc.tile_pool(name="ps_scores", bufs=1, space="PSUM") as ps_scores,
            tc.tile_pool(name="ps_work", bufs=2, space="PSUM") as ps_work,
        ):
            qT_sb = consts.tile([128, DQ // 128, QPC], f32)
            nc.sync.dma_start(qT_sb, qT_ext[:].rearrange("(c p) q -> p c q", p=128))
            kT_sb = consts.tile([128, DK // 128, K], f32)
            nc.sync.dma_start(kT_sb, kT_ext[:].rearrange("(c p) k -> p c k", p=128))
            v_sb = consts.tile([128, K // 128, DV], f32)
            nc.sync.dma_start(v_sb, v_ext[:].rearrange("(c p) d -> p c d", p=128))
            wq_sb = consts.tile([128, DQ // 128, H], f32)
            nc.sync.dma_start(wq_sb, wq_ext[:].rearrange("(c p) h -> p c h", p=128))
            wk_sb = consts.tile([128, DK // 128, H], f32)
            nc.sync.dma_start(wk_sb, wk_ext[:].rearrange("(c p) h -> p c h", p=128))
            zwv_sb = consts.tile([128, HH, 127], bf16)
            nc.sync.dma_start(zwv_sb, zwv_ext[:].rearrange("t p c -> p t c"))
            mask_sb = consts.tile([QPC, K], f32)
            nc.sync.dma_start(mask_sb, mask_ext[:].to_broadcast([QPC, K]))

            ident = consts.tile([QPC, QPC], f32)
            masks.make_identity(nc, ident[:])

            qpT_sb = consts.tile([128, HH, QPC], f32)
            for h in range(HH):
                qp_ps = ps_work.tile([128, QPC], f32, tag="ps_work")
                for c in range(DQ // 128):
                    nc.tensor.matmul(
                        qp_ps,
                        wq_sb[:, c, h * 128 : (h + 1) * 128],
                        qT_sb[:, c, :],
                        start=(c == 0),
                        stop=(c == DQ // 128 - 1),
                    )
                nc.vector.tensor_copy(qpT_sb[:, h, :], qp_ps)

            kpT_sb = consts.tile([128, HH, K], f32)
            for h in range(HH):
                for kh in range(2):
                    kp_ps = ps_work.tile([128, 512], f32, tag="ps_work")
                    for c in range(DK // 128):
                        nc.tensor.matmul(
                            kp_ps,
                            wk_sb[:, c, h * 128 : (h + 1) * 128],
                            kT_sb[:, c, kh * 512 : (kh + 1) * 512],
                            start=(c == 0),
                            stop=(c == DK // 128 - 1),
                        )
                    nc.vector.tensor_copy(kpT_sb[:, h, kh * 512 : (kh + 1) * 512], kp_ps)

            scores_ps = ps_scores.tile([QPC, K], f32)
            for q in range(QPC):
                for h in range(HH):
                    ft = featp.tile([128, K], bf16, tag="feats")
                    nc.scalar.activation(
                        ft,
                        kpT_sb[:, h, :],
                        AF.Tanh,
                        bias=qpT_sb[:, h, q : q + 1],
                        scale=1.0,
                    )
                    win = zwv_sb[:, h, (QPC - 1) - q : (2 * QPC - 1) - q]
                    for kh in range(2):
                        nc.tensor.matmul(
                            scores_ps[:, kh * 512 : (kh + 1) * 512],
                            win,
                            ft[:, kh * 512 : (kh + 1) * 512],
                            start=(q == 0 and h == 0),
                            stop=(q == QPC - 1 and h == HH - 1),
                        )

            scores_sb = consts.tile([QPC, K], f32)
            nc.vector.tensor_add(scores_sb, scores_ps, mask_sb)
            mx = consts.tile([QPC, 1], f32)
            nc.vector.tensor_reduce(
                mx, scores_sb, axis=mybir.AxisListType.X, op=mybir.AluOpType.max,
                negate=True,
            )
            attn_sb = consts.tile([QPC, K], f32)
            esum = consts.tile([QPC, 1], f32)
            nc.scalar.activation(
                attn_sb, scores_sb, AF.Exp, bias=mx[:, 0:1], scale=1.0,
                accum_out=esum[:, 0:1],
            )
            resum = consts.tile([QPC, 1], f32)
            nc.vector.reciprocal(resum, esum)

            attnT_sb = consts.tile([128, K // 128, QPC], f32)
            for t in range(K // 128):
                at_ps = ps_work.tile([128, QPC], f32, tag="ps_work")
                nc.tensor.transpose(at_ps, attn_sb[:, t * 128 : (t + 1) * 128], ident)
                nc.vector.tensor_copy(attnT_sb[:, t, :], at_ps)

            av_ps = ps_work.tile([QPC, DV], f32, tag="ps_av")
            for t in range(K // 128):
                nc.tensor.matmul(
                    av_ps,
                    attnT_sb[:, t, :],
                    v_sb[:, t, :],
                    start=(t == 0),
                    stop=(t == K // 128 - 1),
                )
            out_sb = consts.tile([QPC, DV], f32)
            nc.vector.tensor_scalar_mul(out_sb, av_ps, resum[:, 0:1])
            nc.sync.dma_start(out_ext[:], out_sb)

    nc.finalize()
    return nc


def _slot_order(e4):
    """Group order: second-smallest batch first (short prologue critical
    path), then the heavy batches, smallest last (cheap exposed tail)."""
    d = [int(b) for b in np.argsort([-e for e in e4], kind="stable")]
    if len(d) >= 3:
        d = [d[-2]] + d[: len(d) - 2] + [d[-1]]
    return tuple(d)


def _build_nc_adaptive(e4, vl4, repeat=1):
    """e4[b] = number of 128-wide k-tiles needed for batch b (1..8), vl4[b] =
    exact valid length.  Slots are processed in 4 groups of 16, one batch per
    group (batch order = extent-descending with the first two swapped so the
    prologue waits on a slightly smaller keys DMA and the exposed tail is the
    lightest batch).  A group's rows share one valid length, so softmax runs
    on exactly scores[:, :vl] with no masking, reading PSUM directly."""
    import concourse.mybir as mybir
    from concourse import bacc, masks
    from concourse.tile import TileContext

    f32 = mybir.dt.float32
    bf16 = mybir.dt.bfloat16
    AF = mybir.ActivationFunctionType
    border = _slot_order(e4)  # group s -> batch border[s]
    VLM = max(vl + (vl & 1) for vl in vl4)  # packed k width (even)
    NTM = max((vl + 127) // 128 for vl in vl4)  # packed V chunk count
    # qk/ft pipeline depth from the SBUF budget (large valid_lens shrink it)
    consts_kb = (B * (DK // 128) * VLM * 2 + B * HH * VLM * 2
                 + B * NTM * DV * 2) / 1024 + 24
    qkft_kb = 10 * VLM * 2 / 1024
    QKFT_BUFS = int(max(2, min(4, (186 - consts_kb) // (2 * qkft_kb))))

    nc = bacc.Bacc()

    # Wk alone (kp critical path), then queriesT | Wq packed in one DMA
    wk_ext = nc.declare_dram_parameter("Wk", [DK, H], bf16, isOutput=False)
    qwk_ext = nc.declare_dram_parameter(
        "qwk", [DQ, QPC + H], bf16, isOutput=False
    )
    kT_ext = nc.declare_dram_parameter("keysT4", [B, DK, K], bf16, isOutput=False)
    v_ext = nc.declare_dram_parameter("values4", [B, K, DV], bf16, isOutput=False)
    zwv_ext = nc.declare_dram_parameter(
        "zwv", [HH, 128, 2 * RPB - 1], bf16, isOutput=False
    )
    out_ext = nc.declare_dram_parameter("out", [QPC, DV], f32, isOutput=True)

    with TileContext(nc) as tc:
        with (
            tc.tile_pool(name="consts", bufs=1) as consts,
            tc.tile_pool(name="qk", bufs=QKFT_BUFS) as qkp,
            tc.tile_pool(name="ft", bufs=QKFT_BUFS) as ftp,
            tc.tile_pool(name="epi", bufs=2) as epip,
            tc.tile_pool(name="ps_scores", bufs=2, space="PSUM") as ps_scores,
            tc.tile_pool(name="ps_work", bufs=2, space="PSUM") as ps_work,
            tc.tile_pool(name="ps_av", bufs=2, space="PSUM") as ps_avp,
        ):
            for rep_ in range(repeat):
                W = QPC + H
                wk_sb = consts.tile([128, DK // 128, H], bf16)
                nc.sync.dma_start(wk_sb, wk_ext[:].rearrange("(c p) h -> p c h", p=128))
                kT_sb = consts.tile([128, B, DK // 128, VLM], bf16)
                bf_ = border[0]
                nc.sync.dma_start(
                    kT_sb[:, bf_, :, : vl4[bf_]],
                    kT_ext[bf_].rearrange("(c p) k -> p c k", p=128)[:, :, : vl4[bf_]],
                )
                qwk_sb = consts.tile([128, DQ // 128, W], bf16)
                nc.sync.dma_start(qwk_sb, qwk_ext[:].rearrange("(c p) w -> p c w", p=128))
                qT_sb = qwk_sb[:, :, 0:QPC]
                wq_sb = qwk_sb[:, :, QPC:W]
                zwv_sb = consts.tile([128, HH, 2 * RPB - 1], bf16)
                nc.gpsimd.dma_start(zwv_sb, zwv_ext[:].rearrange("t p c -> p t c"))
                for b in border[1:]:
                    vl = vl4[b]
                    nc.sync.dma_start(
                        kT_sb[:, b, :, :vl],
                        kT_ext[b].rearrange("(c p) k -> p c k", p=128)[:, :, :vl],
                    )
                v_sb = consts.tile([128, B, NTM, DV], bf16)
                for b in border:
                    vl = vl4[b]
                    nfull = vl // 128
                    if nfull:
                        nc.sync.dma_start(
                            v_sb[:, b, :nfull, :],
                            v_ext[b, : nfull * 128, :].rearrange(
                                "(c p) d -> p c d", p=128
                            ),
                        )
                    if vl % 128:
                        nc.sync.dma_start(
                            v_sb[: vl % 128, b, nfull, :],
                            v_ext[b, nfull * 128 : vl, :],
                        )

                ident = consts.tile([RPB, RPB], bf16)
                masks.make_identity(nc, ident[:])

                qpT_sb = consts.tile([128, HH, QPC], f32)
                kpT_sb = consts.tile([128, B, HH, VLM], bf16)

                def qp_proj(h):
                    # qp projection (bf16 in, f32 psum) feeding the adds
                    qp_ps = ps_work.tile(
                        [128, QPC], f32, tag="ps_work", name=f"qp_ps{h}_{rep_}"
                    )
                    for c in range(DQ // 128):
                        nc.tensor.matmul(
                            qp_ps,
                            wq_sb[:, c, h * 128 : (h + 1) * 128],
                            qT_sb[:, c, :],
                            start=(c == 0),
                            stop=(c == DQ // 128 - 1),
                        )
                    nc.vector.tensor_copy(qpT_sb[:, h, :], qp_ps)

                def kp_proj(b, h):
                    # kp projection (bf16 in, f32 psum, bf16 store), k < vl
                    kmax = vl4[b]
                    for k0 in range(0, kmax, 512):
                        kn = min(512, kmax - k0)
                        kp_ps = ps_work.tile(
                            [128, 512], f32, tag="ps_work",
                            name=f"kp_ps{b}_{h}_{k0}_{rep_}",
                        )
                        for c in range(DK // 128):
                            nc.tensor.matmul(
                                kp_ps[:, :kn],
                                wk_sb[:, c, h * 128 : (h + 1) * 128],
                                kT_sb[:, b, c, k0 : k0 + kn],
                                start=(c == 0),
                                stop=(c == DK // 128 - 1),
                            )
                        nc.vector.tensor_copy(
                            kpT_sb[:, b, h, k0 : k0 + kn], kp_ps[:, :kn]
                        )
                    if kmax & 1:
                        # even-pad column so the bf16 qp+kp add can run 4x
                        nc.vector.memset(kpT_sb[:, b, h, kmax : kmax + 1], 0.0)

                # first tanh batch needs only kp(border[0], h0) + qp(h0)
                kp_proj(border[0], 0)
                qp_proj(0)
                kp_proj(border[0], 1)
                qp_proj(1)
                for b in border[1:]:
                    for h in range(HH):
                        kp_proj(b, h)

                TBH = 8  # slot-halves per batched tanh

                def do_group(s):
                    b = border[s]
                    vl = vl4[b]
                    vl2 = vl + (vl & 1)  # even pad for DVE 4x mode
                    nt = (vl + 127) // 128  # k-chunks for transpose/AV
                    scores_ps = ps_scores.tile(
                        [RPB, VLM], f32, tag="scores", name=f"scores_ps{s}_{rep_}"
                    )
                    nc.vector.memset(scores_ps[:, :vl], 0.0)
                    # h-major so the first batch only needs the h0 projections;
                    # small first batch starts ScalarE early, small last batch
                    # keeps the exposed tail short
                    halves = [(jj, h) for h in range(HH) for jj in range(RPB)]
                    bsizes = [2, TBH, TBH, TBH + 2, 32 - 4 - 3 * TBH]
                    bmax = max(bsizes)
                    bstarts = [sum(bsizes[:i]) for i in range(len(bsizes))]
                    for b0, cnt in zip(bstarts, bsizes):
                        # qp+kp adds on VectorE (bf16 4x mode), then one big
                        # bias-free tanh on ScalarE covering `cnt` slot-halves
                        qk = qkp.tile([128, bmax * vl2], bf16, tag="qk")
                        for i in range(cnt):
                            jj, h = halves[b0 + i]
                            j = s * RPB + jj
                            nc.vector.tensor_scalar_add(
                                qk[:, i * vl2 : i * vl2 + vl2],
                                kpT_sb[:, b, h, :vl2],
                                qpT_sb[:, h, j : j + 1],
                            )
                        ft = ftp.tile([128, bmax * vl2], bf16, tag="ft")
                        nc.scalar.activation(
                            ft[:, : cnt * vl2], qk[:, : cnt * vl2], AF.Tanh
                        )
                        for i in range(cnt):
                            jj, h = halves[b0 + i]
                            win = zwv_sb[:, h, (RPB - 1) - jj : (2 * RPB - 1) - jj]
                            for n0 in range(0, vl, 512):
                                nn = min(512, vl - n0)
                                nc.tensor.matmul(
                                    scores_ps[:, n0 : n0 + nn],
                                    win,
                                    ft[:, i * vl2 + n0 : i * vl2 + n0 + nn],
                                    start=False,
                                    stop=(b0 + i == len(halves) - 1),
                                    skip_group_check=True,
                                )

                    # softmax on exactly [:, :vl] straight from PSUM, without
                    # max-subtraction: |score| <= sum|wv| ~ 13, exp is safe in f32
                    attn_sb = epip.tile(
                        [RPB, VLM], bf16, tag="attn", name=f"attn{s}_{rep_}"
                    )
                    esum = epip.tile([RPB, 1], f32, tag="esum", name=f"esum{s}_{rep_}")
                    nc.scalar.activation(
                        attn_sb[:, :vl], scores_ps[:, :vl], AF.Exp,
                        accum_out=esum[:, 0:1],
                    )
                    resum = epip.tile([RPB, 1], f32, tag="resum", name=f"resum{s}_{rep_}")
                    nc.vector.reciprocal(resum, esum)

                    attnT_sb = epip.tile(
                        [128, NTM, RPB], bf16, tag="attnT", name=f"attnT{s}_{rep_}"
                    )
                    for t in range(nt):
                        w = min(128, vl - t * 128)
                        tp_ps = ps_work.tile([128, RPB], bf16, tag="ps_work")
                        nc.tensor.transpose(
                            tp_ps[:w, :], attn_sb[:, t * 128 : t * 128 + w], ident
                        )
                        nc.vector.tensor_copy(attnT_sb[:w, t, :], tp_ps[:w, :])
                    av_ps = ps_avp.tile([RPB, DV], f32, tag="av", name=f"av_ps{s}_{rep_}")
                    for t in range(nt):
                        w = min(128, vl - t * 128)
                        nc.tensor.matmul(
                            av_ps,
                            attnT_sb[:w, t, :],
                            v_sb[:w, b, t, :],
                            start=(t == 0),
                            stop=(t == nt - 1),
                        )
                    out_g = epip.tile(
                        [RPB, DV], f32, tag="out_g", name=f"out_g{s}_{rep_}"
                    )
                    if s == B - 1:
                        # last group: ACT is idle after its exp; skip the DVE hop
                        nc.scalar.activation(out_g, av_ps, AF.Copy, scale=resum[:, 0:1])
                    else:
                        nc.vector.tensor_scalar_mul(out_g, av_ps, resum[:, 0:1])
                    nc.sync.dma_start(out_ext[s * RPB : (s + 1) * RPB, :], out_g)

                for s in range(B):
                    do_group(s)

    nc.finalize()
    return nc


def _extents(valid_lens):
    vl = np.asarray(valid_lens, dtype=np.int64)
    return tuple(int(x) for x in np.clip((vl + 127) // 128, 1, K // 128))


def _make_zwv(wv, width):
    zwv = np.zeros((HH, 128, 2 * width - 1), dtype=np.float32)
    for t in range(HH):
        zwv[t, :, width - 1] = wv[t * 128 : (t + 1) * 128]
    return zwv.astype(ml_dtypes.bfloat16)


def _make_mask_rows(valid_lens):
    return np.where(
        np.arange(K)[None, :] < np.asarray(valid_lens, dtype=np.int64)[:, None],
        np.float32(0.0),
        np.float32(NEG_INF),
    ).astype(np.float32)


def _prep_full(queries, keys, values, valid_lens, Wq, Wk, wv):
    zwv = _make_zwv(wv, QPC)
    mask_rows = _make_mask_rows(valid_lens)
    in_maps = []
    for c in range(N_CORES):
        b = c // (N_CORES // B)
        q0 = (c % (N_CORES // B)) * QPC
        in_maps.append(
            {
                "queriesT": np.ascontiguousarray(queries[b, q0 : q0 + QPC, :].T),
                "keysT": np.ascontiguousarray(keys[b].T),
                "values": np.ascontiguousarray(values[b]),
                "Wq": Wq,
                "Wk": Wk,
                "zwv": zwv,
                "mask": mask_rows[b : b + 1],
            }
        )
    return in_maps


def _prep_adaptive(queries, keys, values, valid_lens, Wq, Wk, wv):
    e4 = _extents(valid_lens)
    border = _slot_order(e4)
    zwv = _make_zwv(wv, RPB)
    keysT4 = np.ascontiguousarray(keys.transpose(0, 2, 1)).astype(ml_dtypes.bfloat16)
    values4 = values.astype(ml_dtypes.bfloat16)
    Wk_bf = Wk.astype(ml_dtypes.bfloat16)
    Wq_bf = Wq.astype(ml_dtypes.bfloat16)
    in_maps = []
    for c in range(N_CORES):
        # slot j: batch border[j//RPB], query row RPB*c + j%RPB
        qT = np.empty((DQ, QPC), np.float32)
        for s in range(B):
            qT[:, s * RPB : (s + 1) * RPB] = queries[
                border[s], RPB * c : RPB * (c + 1), :
            ].T
        qwk = np.concatenate([qT.astype(ml_dtypes.bfloat16), Wq_bf], axis=1)
        in_maps.append(
            {
                "Wk": Wk_bf,
                "qwk": np.ascontiguousarray(qwk),
                "keysT4": keysT4,
                "values4": values4,
                "zwv": zwv,
            }
        )
    return in_maps


def run(inputs: dict, trace: bool = False):
    from concourse.bass_utils import run_bass_kernel_spmd

    queries = np.asarray(inputs["queries"], dtype=np.float32)
    keys = np.asarray(inputs["keys"], dtype=np.float32)
    values = np.asarray(inputs["values"], dtype=np.float32)
    valid_lens = np.asarray(inputs["valid_lens"])
    Wq = np.ascontiguousarray(np.asarray(inputs["Wq"], dtype=np.float32))
    Wk = np.ascontiguousarray(np.asarray(inputs["Wk"], dtype=np.float32))
    wv = np.asarray(inputs["wv"], dtype=np.float32)

    e4 = _extents(valid_lens)
    vl4 = tuple(int(x) for x in np.asarray(valid_lens).reshape(-1))
    adaptive = sum(e4) <= 28
    key = ("adaptive", vl4) if adaptive else ("full",)
    if key not in _CACHE:
        _CACHE[key] = _build_nc_adaptive(e4, vl4) if adaptive else _build_nc_full()
    nc = _CACHE[key]
    prep = _prep_adaptive if adaptive else _prep_full
    in_maps = prep(queries, keys, values, valid_lens, Wq, Wk, wv)
    res = run_bass_kernel_spmd(nc, in_maps, core_ids=list(range(N_CORES)), trace=trace)

    out = np.empty((B, Q, DV), dtype=np.float32)
    border = _slot_order(e4)
    for c in range(N_CORES):
        o = np.asarray(res.results[c]["out"], dtype=np.float32)
        if adaptive:
            for s in range(B):
                out[border[s], RPB * c : RPB * (c + 1), :] = o[
                    s * RPB : (s + 1) * RPB
                ]
        else:
            b = c // (N_CORES // B)
            q0 = (c % (N_CORES // B)) * QPC
            out[b, q0 : q0 + QPC, :] = o
    return out, res.exec_time_ns


def kernel(**inputs) -> np.ndarray:
    out, _ = run(inputs, trace=False)
    return out

